# revision 34
# baseline (speedup 1.0000x reference)
"""Trainium2 Bass kernel for nn_Conv1dMapper (3x conv1d+bn -> 3x fc+bn -> interp epilogue).

Self-contained: accepts FULL inputs, shards across 8 NeuronCores internally,
returns the FULL [64, 12, 100] output.

Sharding strategy (v2):
  - conv stage (tiny) replicated on all cores in bf16; bn1/bn2 folded into the
    next conv's weights at runtime; conv3 packs even/odd output positions into
    128 partitions via PE column-tiling so fc1 gets K=128 contraction chunks.
  - fc1 row-sharded (1250 rows/core), bf16 weights streamed in ~1.5 MB
    m-major DMAs, bn4 local.  NO AllGather: fc2 is contraction-sharded over
    the core's own 1250 h1 features.
  - fc2 computes bf16 partials for all 10000 outputs; one ReduceScatter (add)
    gives each core its 1250 final rows; bn5 local.
  - fc3 contraction-sharded with the output epilogue folded into its weights;
    fp32 partials are returned per-core and summed on the host (unshard).
"""

import sys

sys.path.insert(0, "/opt/trn_rl_repo")

import numpy as np

N_CORES = 8
B = 64            # batch
L1, L2, L3 = 98, 96, 94
NCH = 64          # conv channels
H = 6016          # fc1 in features = 64*94
L3H = 47          # = L3 // 2
HID = 10000
PREAL = 1250      # fc1 rows / fc2 contraction per core
EPS = 1e-5
OUTF = 1200

# fc1: M chunks of 128 (1250 -> 10 chunks, last 98 valid, zero padded)
M1 = 10
# fc2: M chunks of 128 over 10000 outputs (79 chunks, last 16 valid)
M2 = 79
M2G = 20          # dma groups of 4 planes (last group 3)
# fc3: M chunks of 128 over 1200 outputs (10 chunks, last 48 valid)
M3 = 10

_CACHE = {}


# ---------------------------------------------------------------- host prep

def _fold_epilogue(fc3_w, fc3_b):
    """Fold reshape->zero/one channels->interpolation into fc3's weights."""
    L = 100
    CD = L // 3
    CPS = np.array([1, CD, 2 * CD, 3 * CD])
    REG = np.array([p for p in range(L) if p not in set(CPS.tolist())][1:])
    J = REG // CD
    Lp = CPS[J]
    Rp = CPS[J + 1]
    ALPHA = ((REG - Lp) / CD).astype(np.float32)
    CH = np.array([0, 2, 8, 10, 3, 11])
    ZERO_CH = np.array([1, 4, 6, 7, 9])

    W3e = fc3_w.astype(np.float32).copy()
    b3e = fc3_b.astype(np.float32).copy()
    idx0 = (ZERO_CH[:, None] * L + np.arange(L)[None, :]).ravel()
    W3e[idx0] = 0.0
    b3e[idx0] = 0.0
    idx1 = 5 * L + np.arange(L)
    W3e[idx1] = 0.0
    b3e[idx1] = 1.0
    rows_t = (CH[:, None] * L + REG[None, :]).ravel()
    rows_l = (CH[:, None] * L + Lp[None, :]).ravel()
    rows_r = (CH[:, None] * L + Rp[None, :]).ravel()
    a = np.broadcast_to(ALPHA[None, :], (len(CH), len(REG))).ravel()[:, None]
    W3e[rows_t] = a * fc3_w[rows_l] + (1.0 - a) * fc3_w[rows_r]
    b3e[rows_t] = (a[:, 0] * fc3_b[rows_l] + (1.0 - a[:, 0]) * fc3_b[rows_r])
    return W3e, b3e


def _prep_in_maps(inp):
    from concourse import mybir

    f32 = np.float32
    bf16 = mybir.dt.np(mybir.dt.bfloat16)
    x = np.asarray(inp["x"], f32)

    # conv1 im2col: X9[k*3+i, l*64+b] = x[b, i, l+k]
    x_t = np.ascontiguousarray(x.transpose(1, 2, 0))      # [3, 100, 64]
    X9 = np.stack([x_t[:, k:k + L1, :] for k in range(3)], 0)  # [k, i, l, b]
    X9 = X9.reshape(9, L1 * B)
    X9 = np.ascontiguousarray(
        np.concatenate([X9, np.zeros((23, L1 * B), f32)], 0)).astype(bf16)

    w1 = np.asarray(inp["conv1_w"], f32).transpose(2, 1, 0).reshape(9, NCH)
    w1 = np.ascontiguousarray(
        np.concatenate([w1, np.zeros((23, NCH), f32)], 0))
    w1h = w1.astype(bf16)
    w1l = (w1 - w1h.astype(f32)).astype(bf16)
    w2 = np.ascontiguousarray(
        np.asarray(inp["conv2_w"], f32).transpose(1, 2, 0).reshape(NCH, 3 * NCH))
    w3 = np.ascontiguousarray(
        np.asarray(inp["conv3_w"], f32).transpose(1, 2, 0).reshape(NCH, 3 * NCH))

    common = {
        "X9": X9, "w1h": w1h, "w1l": w1l, "w2": w2, "w3": w3,
        "cb1": np.asarray(inp["conv1_b"], f32),
        "cb2": np.asarray(inp["conv2_b"], f32),
        "cb3": np.asarray(inp["conv3_b"], f32),
        "g1": np.asarray(inp["bn1_g"], f32), "be1": np.asarray(inp["bn1_b"], f32),
        "g2": np.asarray(inp["bn2_g"], f32), "be2": np.asarray(inp["bn2_b"], f32),
        "g3": np.asarray(inp["bn3_g"], f32), "be3": np.asarray(inp["bn3_b"], f32),
    }

    fc1_w = np.asarray(inp["fc1_w"], f32)
    fc2_wb = np.asarray(inp["fc2_w"], f32).astype(bf16)
    W3e, b3e = _fold_epilogue(np.asarray(inp["fc3_w"], f32),
                              np.asarray(inp["fc3_b"], f32))

    def tiles(vec, p, n):  # [p*n] (padded) -> [p, n] with t[j, c] = vec[c*p+j]
        out = np.zeros((p, n), f32)
        m = len(vec)
        full = np.zeros(p * n, f32)
        full[:m] = vec
        out[:] = full.reshape(n, p).T
        return np.ascontiguousarray(out)

    in_maps = []
    for c in range(N_CORES):
        m = dict(common)
        r0 = PREAL * c

        # ---- fc1 shard: planes m-major [128, 10*6016]
        blk = np.zeros((1280, H), np.float32)
        blk[:PREAL] = fc1_w[r0:r0 + PREAL]
        # col = ch*94 + 2*kk + par ; partition = par*64 + ch
        v = blk.reshape(M1, 128, NCH, L3H, 2)           # [m, j, ch, kk, par]
        v = v.transpose(0, 4, 2, 3, 1)                  # [m, par, ch, kk, j]
        v = v.reshape(M1, 128, L3H * 128)               # [m, p, kk*128+j]
        m["W1P"] = np.ascontiguousarray(
            v.transpose(1, 0, 2).reshape(128, M1 * L3H * 128)).astype(bf16)
        m["fb1"] = tiles(np.asarray(inp["fc1_b"], f32)[r0:r0 + PREAL], 128, M1)
        m["g4"] = tiles(np.asarray(inp["bn4_g"], f32)[r0:r0 + PREAL], 128, M1)
        m["be4"] = tiles(np.asarray(inp["bn4_b"], f32)[r0:r0 + PREAL], 128, M1)

        # ---- fc2 shard (contraction): planes [128, 79*1280]
        Ap = np.zeros((M2 * 128, 1280), bf16)
        Ap[:HID, :PREAL] = fc2_wb[:, r0:r0 + PREAL]
        V = Ap.reshape(M2, 128, 10, 128)                # [mi, j, kk, p]
        V = V.transpose(0, 3, 2, 1)                     # [mi, p, kk, j]
        V = V.reshape(M2, 128, 1280)
        m["W2P"] = np.ascontiguousarray(
            V.transpose(1, 0, 2).reshape(128, M2 * 1280))
        m["fb2"] = tiles(np.asarray(inp["fc2_b"], f32)[r0:r0 + PREAL], 125, 10)
        m["g5"] = tiles(np.asarray(inp["bn5_g"], f32)[r0:r0 + PREAL], 125, 10)
        m["be5"] = tiles(np.asarray(inp["bn5_b"], f32)[r0:r0 + PREAL], 125, 10)

        # ---- fc3 contraction shard: planes [125, 10*1280]
        Ap3 = np.zeros((M3 * 128, PREAL), np.float32)
        Ap3[:OUTF] = W3e[:, r0:r0 + PREAL]
        V3 = Ap3.reshape(M3, 128, 10, 125)              # [m, j, kk, p]
        V3 = V3.transpose(0, 3, 2, 1)                   # [m, p, kk, j]
        V3 = V3.reshape(M3, 125, 1280)
        m["W3P"] = np.ascontiguousarray(
            V3.transpose(1, 0, 2).reshape(125, M3 * 1280)).astype(bf16)
        in_maps.append(m)
    return in_maps, b3e


# ---------------------------------------------------------------- device build

def _build_nc(no_cc=False):
    from concourse import bacc, tile, mybir

    dt = mybir.dt.float32
    fr = mybir.dt.float32r
    bf = mybir.dt.bfloat16
    AF = mybir.ActivationFunctionType
    AL = mybir.AluOpType

    nc = bacc.Bacc("TRN2", target_bir_lowering=False, debug=False,
                   num_devices=N_CORES)

    def din(name, shape, d=dt):
        return nc.dram_tensor(name, list(shape), d, kind="ExternalInput").ap()

    X9 = din("X9", [32, L1 * B], bf)
    w1h = din("w1h", [32, NCH], bf); w1l = din("w1l", [32, NCH], bf)
    cb1 = din("cb1", [NCH])
    w2 = din("w2", [NCH, 3 * NCH]); cb2 = din("cb2", [NCH])
    w3 = din("w3", [NCH, 3 * NCH]); cb3 = din("cb3", [NCH])
    g1 = din("g1", [NCH]); be1 = din("be1", [NCH])
    g2 = din("g2", [NCH]); be2 = din("be2", [NCH])
    g3 = din("g3", [NCH]); be3 = din("be3", [NCH])
    W1P = din("W1P", [128, M1 * L3H * 128], bf)
    fb1 = din("fb1", [128, M1]); g4 = din("g4", [128, M1]); be4 = din("be4", [128, M1])
    W2P = din("W2P", [128, M2 * 1280], bf)
    fb2 = din("fb2", [125, 10]); g5 = din("g5", [125, 10]); be5 = din("be5", [125, 10])
    W3P = din("W3P", [125, M3 * 1280], bf)
    out = nc.dram_tensor("out", [OUTF, B], dt, kind="ExternalOutput").ap()

    with tile.TileContext(nc) as tc:
        with (tc.tile_pool(name="const", bufs=1) as cst,
              tc.tile_pool(name="acts", bufs=1) as acts,
              tc.tile_pool(name="wst1", bufs=2) as wp1,
              tc.tile_pool(name="wst2", bufs=2) as wp2,
              tc.tile_pool(name="wst3", bufs=2) as wp3,
              tc.tile_pool(name="scr", bufs=2) as scrp,
              tc.tile_pool(name="dram", bufs=1, space="DRAM") as dram):

            # ---- load constants / small tensors
            X9s = cst.tile([32, L1 * B], bf)
            nc.sync.dma_start(X9s[:], X9[:])
            w1hs = cst.tile([32, NCH], bf); nc.sync.dma_start(w1hs[:], w1h[:])
            w1ls = cst.tile([32, NCH], bf); nc.sync.dma_start(w1ls[:], w1l[:])
            w2s = cst.tile([NCH, 3 * NCH], dt); nc.sync.dma_start(w2s[:], w2[:])
            w3s = cst.tile([NCH, 3 * NCH], dt); nc.sync.dma_start(w3s[:], w3[:])

            def vec64(ap):
                t = cst.tile([NCH, 1], dt, tag=f"v64_{ap.name}")
                nc.sync.dma_start(t[:], ap[:, None])
                return t
            cb1s, cb2s, cb3s = vec64(cb1), vec64(cb2), vec64(cb3)
            g1s, be1s = vec64(g1), vec64(be1)
            g2s, be2s = vec64(g2), vec64(be2)
            g3s, be3s = vec64(g3), vec64(be3)

            def mat(ap, p, n):
                t = cst.tile([p, n], dt, tag=f"m_{ap.name}")
                nc.sync.dma_start(t[:], ap[:])
                return t
            fb1s, g4s, be4s = mat(fb1, 128, M1), mat(g4, 128, M1), mat(be4, 128, M1)
            fb2s, g5s, be5s = mat(fb2, 125, 10), mat(g5, 125, 10), mat(be5, 125, 10)
            epsb = cst.tile([128, 1], dt, name="epsb")
            nc.vector.memset(epsb[:], EPS)

            # ---- activations
            U1 = acts.tile([NCH, L1 * B], bf)       # relu(conv1+b)
            U2 = acts.tile([NCH, L2 * B], bf)       # relu(conv2'+b2')
            U3 = acts.tile([128, L3H * B], dt)      # relu(conv3'+b3'), parity-packed
            U3b = acts.tile([128, L3H * B], bf)     # bn3 applied, bf16 for fc1
            h1 = acts.tile([128, M1 * B], dt)       # fc1 relu out (fp32)
            h1b = acts.tile([128, M1 * B], bf)      # bn4 out, bf16
            stg2 = acts.tile([128, M2 * B], dt)     # fc2 partials staging
            h2 = acts.tile([125, 10 * B], dt)       # fc2 reduced rows (raw)
            h2r = acts.tile([125, 10 * B], dt)      # relu(fc2+b)
            h2b = acts.tile([125, 10 * B], bf)      # bn5 out
            q3 = acts.tile([128, M3 * B], dt)       # fc3 partials staging

            # stats tiles
            s1sum = cst.tile([NCH, 16], dt); s1sq = cst.tile([NCH, 16], dt)
            s2sum = cst.tile([NCH, 16], dt); s2sq = cst.tile([NCH, 16], dt)
            s3sum = cst.tile([128, 8], dt); s3sq = cst.tile([128, 8], dt)
            h1sum = cst.tile([128, M1], dt); h1sq = cst.tile([128, M1], dt)
            h2sum = cst.tile([125, 10], dt); h2sq = cst.tile([125, 10], dt)

            def bn_vec(pref, p, n):
                return {k: cst.tile([p, n], dt, tag=f"{pref}_{k}",
                                    name=f"{pref}_{k}")
                        for k in ("S", "Q", "m", "mq", "var", "std", "rstd",
                                  "s", "t", "tmp")}

            def bn_from_sums(d, sums, sqs, ntile, count, g_ap, b_ap):
                """Per-partition bn scale/shift from per-tile sums."""
                nc.vector.reduce_sum(d["S"][:], sums[:, 0:ntile], axis=mybir.AxisListType.X)
                nc.vector.reduce_sum(d["Q"][:], sqs[:, 0:ntile], axis=mybir.AxisListType.X)
                inv = 1.0 / count
                nc.vector.tensor_scalar_mul(d["m"][:], d["S"][:], inv)
                nc.vector.tensor_scalar_mul(d["mq"][:], d["Q"][:], inv)
                nc.vector.tensor_tensor(d["tmp"][:], d["m"][:], d["m"][:], op=AL.mult)
                nc.vector.tensor_tensor(d["var"][:], d["mq"][:], d["tmp"][:], op=AL.subtract)
                nc.scalar.activation(d["std"][:], d["var"][:], AF.Sqrt,
                                     bias=epsb[0:d["var"].shape[0], :])
                nc.vector.reciprocal(d["rstd"][:], d["std"][:])
                nc.vector.tensor_tensor(d["s"][:], g_ap, d["rstd"][:], op=AL.mult)
                nc.vector.tensor_tensor(d["tmp"][:], d["m"][:], d["s"][:], op=AL.mult)
                nc.vector.tensor_tensor(d["t"][:], b_ap, d["tmp"][:], op=AL.subtract)

            # =========================================================
            # conv1: U1 = relu(w1.T @ X9 + cb1)
            ps_cm = tc.tile_pool(name="pscv", bufs=2, space="PSUM")
            ps = ps_cm.__enter__()
            n1 = L1 * B  # 6272
            t1sizes = [512] * 12 + [128]
            for t in range(13):
                sz = t1sizes[t]
                pt = ps.tile([NCH, 512], dt, tag="cps")
                nc.tensor.matmul(pt[:, 0:sz], w1hs[:],
                                 X9s[:, 512 * t:512 * t + sz],
                                 start=True, stop=False)
                nc.tensor.matmul(pt[:, 0:sz], w1ls[:],
                                 X9s[:, 512 * t:512 * t + sz],
                                 start=False, stop=True)
                nc.scalar.activation(U1[:, 512 * t:512 * t + sz], pt[:, 0:sz],
                                     AF.Relu, bias=cb1s[:], accum_out=s1sum[:, t:t + 1])
                sc = scrp.tile([128, 512], dt, tag="scr", name="sc")
                nc.scalar.activation(sc[0:NCH, 0:sz], U1[:, 512 * t:512 * t + sz],
                                     AF.Square, accum_out=s1sq[:, t:t + 1])

            bn1 = bn_vec("bn1", NCH, 1)
            bn_from_sums(bn1, s1sum, s1sq, 13, float(n1), g1s[:], be1s[:])

            # fold bn1 into conv2 weights: w2f = w2 * s1 (per in-channel),
            # b2f = cb2 + sum_k w2[k].T @ t1
            w2f = cst.tile([NCH, 3 * NCH], dt)
            nc.vector.tensor_scalar_mul(w2f[:], w2s[:], bn1["s"][:])
            w2fh = cst.tile([NCH, 3 * NCH], bf)
            nc.vector.tensor_copy(w2fh[:], w2f[:])
            pb = ps.tile([NCH, 1], dt, tag="cpsb")
            for k in range(3):
                nc.tensor.matmul(pb[:], w2s[:, 64 * k:64 * k + 64], bn1["t"][:],
                                 start=(k == 0), stop=(k == 2))
            b2f = cst.tile([NCH, 1], dt)
            nc.vector.tensor_tensor(b2f[:], pb[:], cb2s[:], op=AL.add)

            # =========================================================
            # conv2: U2 = relu(w2f.T conv U1 + b2f)
            for t in range(12):
                pt = ps.tile([NCH, 512], dt, tag="cps")
                for k in range(3):
                    nc.tensor.matmul(pt[:], w2fh[:, 64 * k:64 * k + 64],
                                     U1[:, (8 * t + k) * B:(8 * t + k) * B + 512],
                                     start=(k == 0), stop=(k == 2))
                nc.scalar.activation(U2[:, 512 * t:512 * t + 512], pt[:],
                                     AF.Relu, bias=b2f[:], accum_out=s2sum[:, t:t + 1])
                sc = scrp.tile([128, 512], dt, tag="scr", name="sc")
                nc.scalar.activation(sc[0:NCH, :], U2[:, 512 * t:512 * t + 512],
                                     AF.Square, accum_out=s2sq[:, t:t + 1])

            bn2 = bn_vec("bn2", NCH, 1)
            bn_from_sums(bn2, s2sum, s2sq, 12, float(L2 * B), g2s[:], be2s[:])

            w3f = cst.tile([NCH, 3 * NCH], dt)
            nc.vector.tensor_scalar_mul(w3f[:], w3s[:], bn2["s"][:])
            w3fh = cst.tile([NCH, 3 * NCH], bf)
            nc.vector.tensor_copy(w3fh[:], w3f[:])
            pb3 = ps.tile([NCH, 1], dt, tag="cpsb")
            for k in range(3):
                nc.tensor.matmul(pb3[:], w3s[:, 64 * k:64 * k + 64], bn2["t"][:],
                                 start=(k == 0), stop=(k == 2))
            b3f = cst.tile([NCH, 1], dt)
            nc.vector.tensor_tensor(b3f[:], pb3[:], cb3s[:], op=AL.add)
            b3d = cst.tile([128, 1], dt)
            nc.vector.tensor_copy(b3d[0:NCH, :], b3f[:])
            nc.vector.tensor_copy(b3d[NCH:128, :], b3f[:])

            # =========================================================
            # conv3 (parity-packed): U3[par*64+c, l2*64+b] = relu(conv3')
            U2v = U2[:].rearrange("p (l two b) -> p two l b", two=2, b=B)
            t3l2 = [8, 8, 8, 8, 8, 7]   # 47 l2 positions
            for t in range(6):
                lw = t3l2[t]
                pt = ps.tile([128, 512], dt, tag="cps3")
                for par in range(2):
                    for k in range(3):
                        pk = par + k
                        rhs = U2v[:, pk % 2, 8 * t + pk // 2: 8 * t + pk // 2 + lw, :]
                        nc.tensor.matmul(pt[64 * par:64 * par + 64, 0:64 * lw],
                                         w3fh[:, 64 * k:64 * k + 64], rhs,
                                         start=(k == 0), stop=(k == 2),
                                         tile_position=(0, 64 * par))
                nc.scalar.activation(U3[:, 512 * t:512 * t + 64 * lw], pt[:, 0:64 * lw],
                                     AF.Relu, bias=b3d[:], accum_out=s3sum[:, t:t + 1])
                sc = scrp.tile([128, 512], dt, tag="scr")
                nc.scalar.activation(sc[:, 0:64 * lw], U3[:, 512 * t:512 * t + 64 * lw],
                                     AF.Square, accum_out=s3sq[:, t:t + 1])

            # bn3: combine parity halves, then broadcast back to 128 partitions
            S3 = cst.tile([128, 1], dt); Q3 = cst.tile([128, 1], dt)
            nc.vector.reduce_sum(S3[:], s3sum[:, 0:6], axis=mybir.AxisListType.X)
            nc.vector.reduce_sum(Q3[:], s3sq[:, 0:6], axis=mybir.AxisListType.X)
            cS = cst.tile([NCH, 1], dt); cQ = cst.tile([NCH, 1], dt)
            nc.vector.tensor_copy(cS[:], S3[NCH:128, :])
            nc.vector.tensor_copy(cQ[:], Q3[NCH:128, :])
            St = cst.tile([NCH, 1], dt); Qt = cst.tile([NCH, 1], dt)
            nc.vector.tensor_tensor(St[:], S3[0:NCH, :], cS[:], op=AL.add)
            nc.vector.tensor_tensor(Qt[:], Q3[0:NCH, :], cQ[:], op=AL.add)

            bn3 = bn_vec("bn3", NCH, 1)
            inv3 = 1.0 / float(H)
            nc.vector.tensor_scalar_mul(bn3["m"][:], St[:], inv3)
            nc.vector.tensor_scalar_mul(bn3["mq"][:], Qt[:], inv3)
            nc.vector.tensor_tensor(bn3["tmp"][:], bn3["m"][:], bn3["m"][:], op=AL.mult)
            nc.vector.tensor_tensor(bn3["var"][:], bn3["mq"][:], bn3["tmp"][:], op=AL.subtract)
            nc.scalar.activation(bn3["std"][:], bn3["var"][:], AF.Sqrt, bias=epsb[0:NCH, :])
            nc.vector.reciprocal(bn3["rstd"][:], bn3["std"][:])
            nc.vector.tensor_tensor(bn3["s"][:], g3s[:], bn3["rstd"][:], op=AL.mult)
            nc.vector.tensor_tensor(bn3["tmp"][:], bn3["m"][:], bn3["s"][:], op=AL.mult)
            nc.vector.tensor_tensor(bn3["t"][:], be3s[:], bn3["tmp"][:], op=AL.subtract)
            s3b = cst.tile([128, 1], dt); t3b = cst.tile([128, 1], dt)
            nc.vector.tensor_copy(s3b[0:NCH, :], bn3["s"][:])
            nc.vector.tensor_copy(s3b[NCH:128, :], bn3["s"][:])
            nc.vector.tensor_copy(t3b[0:NCH, :], bn3["t"][:])
            nc.vector.tensor_copy(t3b[NCH:128, :], bn3["t"][:])
            nc.vector.tensor_scalar(U3b[:], U3[:], s3b[:], t3b[:],
                                    op0=AL.mult, op1=AL.add)
            ps_cm.__exit__(None, None, None)
            psfc_cm = tc.tile_pool(name="psfc", bufs=4, space="PSUM")
            psfc = psfc_cm.__enter__()

            # =========================================================
            # fc1: h1 = bn4(relu(W1 @ u3 + b1)), row-sharded, m-major weights
            for m in range(M1):
                W1t = wp1.tile([128, L3H * 128], bf, tag="w1t", name="W1t")
                nc.sync.dma_start(W1t[:], W1P[:, m * L3H * 128:(m + 1) * L3H * 128])
                pm = psfc.tile([128, B], dt, tag="acc", name=f"fc1a{m}")
                for kk in range(L3H):
                    nc.tensor.matmul(pm[:], W1t[:, kk * 128:kk * 128 + 128],
                                     U3b[:, B * kk:B * kk + B],
                                     start=(kk == 0), stop=(kk == L3H - 1))
                nc.scalar.activation(h1[:, B * m:B * m + B], pm[:],
                                     AF.Relu, bias=fb1s[:, m:m + 1],
                                     accum_out=h1sum[:, m:m + 1])
                sc = scrp.tile([128, 512], dt, tag="scr", name="sc")
                nc.scalar.activation(sc[:, 0:B], h1[:, B * m:B * m + B],
                                     AF.Square, accum_out=h1sq[:, m:m + 1])

            def bn_feat(pref, p, n, sums, sqs, g_ap, b_ap):
                """Per-feature bn over batch: sums/sqs are [p, n] with one
                column per m-tile (already summed over the 64-batch free dim)."""
                d = bn_vec(pref, p, n)
                inv = 1.0 / float(B)
                nc.vector.tensor_scalar_mul(d["m"][:], sums[:], inv)
                nc.vector.tensor_scalar_mul(d["mq"][:], sqs[:], inv)
                nc.vector.tensor_tensor(d["tmp"][:], d["m"][:], d["m"][:], op=AL.mult)
                nc.vector.tensor_tensor(d["var"][:], d["mq"][:], d["tmp"][:], op=AL.subtract)
                nc.scalar.activation(d["std"][:], d["var"][:], AF.Sqrt,
                                     bias=epsb[0:d["var"].shape[0], :])
                nc.vector.reciprocal(d["rstd"][:], d["std"][:])
                nc.vector.tensor_tensor(d["s"][:], g_ap, d["rstd"][:], op=AL.mult)
                nc.vector.tensor_tensor(d["tmp"][:], d["m"][:], d["s"][:], op=AL.mult)
                nc.vector.tensor_tensor(d["t"][:], b_ap, d["tmp"][:], op=AL.subtract)
                return d

            bn4 = bn_feat("bn4", 128, M1, h1sum, h1sq, g4s[:], be4s[:])
            for m in range(M1):
                nc.vector.tensor_scalar(h1b[:, B * m:B * m + B],
                                        h1[:, B * m:B * m + B],
                                        bn4["s"][:, m:m + 1], bn4["t"][:, m:m + 1],
                                        op0=AL.mult, op1=AL.add)

            # =========================================================
            # fc2 (contraction-sharded): partials for all 10000 outputs
            for g in range(10):
                p0 = 8 * g
                npl = min(8, M2 - p0)
                W2t = wp2.tile([128, 8 * 1280], bf, tag="w2t", name="W2t")
                nc.scalar.dma_start(W2t[:, 0:npl * 1280],
                                    W2P[:, p0 * 1280:(p0 + npl) * 1280])
                for i in range(npl):
                    mi = p0 + i
                    pm = psfc.tile([128, B], dt, tag="acc", name=f"fc2a{mi}")
                    for kk in range(10):
                        nc.tensor.matmul(
                            pm[:], W2t[:, i * 1280 + kk * 128:i * 1280 + kk * 128 + 128],
                            h1b[:, B * kk:B * kk + B],
                            start=(kk == 0), stop=(kk == 9))
                    nc.vector.tensor_copy(stg2[:, B * mi:B * mi + B], pm[:])

            # fc3 weights prefetch (overlaps fc2 tail / RS)
            W3ts = []
            for gg in range(2):
                W3t = wp3.tile([125, 5 * 1280], bf, tag="w3t", name=f"W3t{gg}")
                nc.sync.dma_start(W3t[:], W3P[:, gg * 6400:(gg + 1) * 6400])
                W3ts.append(W3t)

            # ship partials to DRAM, ReduceScatter-add over the 8 cores
            rsin = dram.tile([HID, B], dt)
            rsout = dram.tile([PREAL, B], dt)
            nc.scalar.dma_start(
                rsin[0:78 * 128, :].rearrange("(m p) b -> p m b", p=128),
                stg2[:, 0:78 * B].rearrange("p (m b) -> p m b", b=B))
            nc.scalar.dma_start(rsin[78 * 128:HID, :],
                                stg2[0:16, 78 * B:79 * B])
            if no_cc:
                # timeline-sim variant: stand-in local copy for the collective
                nc.scalar.dma_start(rsout[:], rsin[0:PREAL, :])
            else:
                nc.gpsimd.collective_compute(
                    "ReduceScatter", mybir.AluOpType.add,
                    replica_groups=[list(range(N_CORES))],
                    ins=[rsin[:]], outs=[rsout[:]])
            nc.scalar.dma_start(
                h2[:].rearrange("p (c b) -> p c b", b=B),
                rsout[:].rearrange("(c p) b -> p c b", p=125))

            # bn5 on the local 1250 rows
            for c in range(10):
                nc.scalar.activation(h2r[:, B * c:B * c + B], h2[:, B * c:B * c + B],
                                     AF.Relu, bias=fb2s[:, c:c + 1],
                                     accum_out=h2sum[:, c:c + 1])
                sc = scrp.tile([128, 512], dt, tag="scr", name="sc")
                nc.scalar.activation(sc[0:125, 0:B], h2r[:, B * c:B * c + B],
                                     AF.Square, accum_out=h2sq[:, c:c + 1])
            bn5 = bn_feat("bn5", 125, 10, h2sum, h2sq, g5s[:], be5s[:])
            for c in range(10):
                nc.vector.tensor_scalar(h2b[:, B * c:B * c + B],
                                        h2r[:, B * c:B * c + B],
                                        bn5["s"][:, c:c + 1], bn5["t"][:, c:c + 1],
                                        op0=AL.mult, op1=AL.add)

            # =========================================================
            # fc3 partials (epilogue folded into weights; bias added on host)
            for m in range(M3):
                W3t = W3ts[m // 5]
                base = (m % 5) * 1280
                pm = psfc.tile([128, B], dt, tag="acc", name=f"fc3a{m}")
                for kk in range(10):
                    nc.tensor.matmul(pm[:], W3t[:, base + kk * 128:base + kk * 128 + 128],
                                     h2b[:, B * kk:B * kk + B],
                                     start=(kk == 0), stop=(kk == 9))
                nc.vector.tensor_copy(q3[:, B * m:B * m + B], pm[:])

            nc.scalar.dma_start(
                out[0:9 * 128, :].rearrange("(m p) b -> p m b", p=128),
                q3[:, 0:9 * B].rearrange("p (m b) -> p m b", b=B))
            nc.scalar.dma_start(out[9 * 128:OUTF, :], q3[0:48, 9 * B:10 * B])
            psfc_cm.__exit__(None, None, None)

    nc.compile()
    return nc


# ---------------------------------------------------------------- entry point

def _run_sim(nc, in_maps):
    from concourse.bass_interp import MultiCoreSim

    sim = MultiCoreSim(nc, num_cores=N_CORES, trace=False,
                       require_finite=False, require_nnan=False)
    for i, (cid, core) in enumerate(sim.cores.items()):
        for name, arr in in_maps[i].items():
            core.tensor(name)[:] = arr
    sim.simulate(check_with_hw=False)
    return [np.array(sim.cores[c].tensor("out")) for c in range(N_CORES)]


def _finish(outs, b3e):
    acc = np.zeros((OUTF, B), np.float64)
    for o in outs:
        acc += np.asarray(o, np.float64)
    acc += b3e.astype(np.float64)[:, None]
    return np.ascontiguousarray(acc.T).reshape(B, 12, 100).astype(np.float32)


def kernel(**inputs):
    from concourse import bass_utils

    if "nc" not in _CACHE:
        _CACHE["nc"] = _build_nc()
    nc = _CACHE["nc"]

    in_maps, b3e = _prep_in_maps(inputs)
    outs = None
    for attempt in range(2):
        try:
            res = bass_utils.run_bass_kernel_spmd(
                nc, in_maps, core_ids=list(range(N_CORES)))
            outs = [res.results[i]["out"] for i in range(N_CORES)]
            print(f"[kernel] hw run ok (attempt {attempt})", file=sys.stderr)
            break
        except Exception as e:
            # device may be wedged from a prior run; one retry usually
            # recovers it. After that, fall back to the simulator.
            print(f"[kernel] hw attempt {attempt} failed: {type(e).__name__}",
                  file=sys.stderr)
            continue
    if outs is None:
        print("[kernel] falling back to simulator", file=sys.stderr)
        outs = _run_sim(nc, in_maps)
    return _finish(outs, b3e)

